# revision 30
# baseline (speedup 1.0000x reference)
"""Trainium2 Bass kernel for GQA attention (B=8, S=1024, H=2048, 32 Q / 8 KV heads, D=64).

Data-parallel over batch: one batch element per NeuronCore, weights replicated,
zero collectives. Host-side prep (numpy): hidden transposed to [H, S] and
decomposed into fp8e4 hi+lo at scale 16; Wq/Wk/Wv decomposed into fp8e4 hi+lo
at scale 512; Wo cast bf16; RoPE tables pre-scaled by 1/(16*512).

Device pipeline per core:
  1. Q/K/V projections as fp8 DoubleRow matmuls (2 K-chunks per pass,
     0.5 cycles/row), 3 error-compensated terms: Hh*Wh + Hh*Wl + Hl*Wh.
  2. RoPE via partition-shift SBUF DMAs + DVE/Pool mul-add (tables carry the
     fp8 descale), q in a rolling SBUF pool bf16, k duplicated into both
     64-partition slots of kT.
  3. Attention per head in scoresT [keys, queries] layout: causal-range QK
     matmuls with the diagonal NEG mask accumulated via identity matmul,
     one exp per key-tile on ScalarE (only Act work), PV in [query, d]
     layout (65-col matmuls, ones-column denominator), per-partition
     normalization (reciprocal + tensor_scalar_mul), PE-transpose back to
     [d, query] into attT bf16 (transpose PSUM borrowed from the wide
     score pool via bitcast).
  4. Software pipelining: step i runs attention(i-1) with Q-proj(i+1)
     DoubleRow matmuls as head-start PE filler and QK emitted two key-tiles
     ahead of PV; transposes(i-2) at step front; Wq streamed per-step; Wo
     prefetched during the last steps.
  5. O-projection bf16 from attT.
"""

import contextlib

import numpy as np
import ml_dtypes

import concourse.bass as bass
import concourse.tile as tile
from concourse import bacc, mybir
from concourse.bass_utils import run_bass_kernel_spmd

B, S, H = 8, 1024, 2048
NQ, NKV, D = 32, 8, 64
F32 = mybir.dt.float32
BF16 = mybir.dt.bfloat16
F8 = mybir.dt.float8e4
DR = mybir.MatmulPerfMode.DoubleRow
AF = mybir.ActivationFunctionType
WS = 512.0  # fp8 weight scale
HS = 16.0  # fp8 hidden scale
E4 = ml_dtypes.float8_e4m3
BF = ml_dtypes.bfloat16


def _host_tables():
    inv = 1.0 / (10000.0 ** (np.arange(0, D, 2, dtype=np.float64) / D))  # [32]
    fr = np.arange(S, dtype=np.float64)[:, None] * inv[None, :]  # [S, 32]
    cos = np.cos(fr).T  # [32, S]
    sin = np.sin(fr).T
    cosT = np.concatenate([cos, cos], 0)  # [64, S]
    sgnT = np.concatenate([-sin, sin], 0)  # [64, S]
    cos128 = (np.concatenate([cosT, cosT], 0) / (WS * HS)).astype(np.float32)
    sgn128 = (np.concatenate([sgnT, sgnT], 0) / (WS * HS)).astype(np.float32)
    p = np.arange(128)[:, None]  # key row
    c = np.arange(128)[None, :]  # query col
    tri = np.where(c >= p, 0.0, -1.0e30).astype(np.float32).astype(BF)  # additive mask
    ident = np.eye(128, dtype=np.float32).astype(BF)
    return cos128, sgn128, tri, ident


def _rope(nc, rp, ps, cos_sl, sgn_sl, out_sl, cp_eng=None):
    """psum [128,512] (scaled qT/kT tile) -> RoPE applied, written to out_sl (bf16).

    GPSIMD cannot access PSUM, so the raw copy runs on cp_eng (DVE/Act)."""
    raw = rp.tile([128, 512], F32, name="rope_raw", tag="rope_raw")
    if cp_eng is None:
        nc.vector.tensor_copy(raw[:], ps[:])
    else:
        cp_eng.copy(raw[:], ps[:])
    sh = rp.tile([128, 512], F32, name="rope_sh", tag="rope_sh")
    for a in range(4):  # partition quarter a reads quarter a^1 (p -> p xor 32)
        sc = (a ^ 1) * 32
        eng = nc.sync if a % 2 == 0 else nc.gpsimd
        eng.dma_start(out=sh[a * 32 : (a + 1) * 32, :], in_=raw[sc : sc + 32, :])
    tmp = rp.tile([128, 512], F32, name="rope_tmp", tag="rope_tmp")
    nc.vector.tensor_mul(tmp[:], raw[:], cos_sl)
    rot = rp.tile([128, 512], F32, name="rope_rot", tag="rope_rot")
    nc.gpsimd.tensor_mul(rot[:], sh[:], sgn_sl)
    nc.vector.tensor_add(out_sl, tmp[:], rot[:])


# DoubleRow 3-term schedule: (hidden term, weight term) with hi=0, lo=1.
# Ordered so hi-only terms run first (their DMAs land earlier).
TERMS = [(0, 0), (1, 0), (0, 1)]


def _body(nc, tc, ctx, tensors):
    (hth, htl, wqh, wql, wkh, wkl, wvh, wvl, wob, cosd, sgnd, trid, identd, outd) = tensors

    # ---- constants (tiles now, DMAs emitted after the wv prefetch) ----
    cpool = ctx.enter_context(tc.tile_pool(name="const", bufs=1))
    tri_t = cpool.tile([128, 128], BF16, name="tri", tag="tri")
    ident_t = cpool.tile([128, 128], BF16, name="ident", tag="ident")
    cos_t = cpool.tile([128, S], F32, name="cos", tag="cos")
    sgn_t = cpool.tile([128, S], F32, name="sgn", tag="sgn")

    # ---- persistent SBUF tensors ----
    attp = ctx.enter_context(tc.tile_pool(name="attTp", bufs=1, side="right"))
    attT = attp.tile([128, 16 * S], BF16, name="attT", tag="attT")
    wop = ctx.enter_context(tc.tile_pool(name="wo", bufs=2))

    # everything below `mid` is freed before the O projection
    mid = contextlib.ExitStack()
    hpool = mid.enter_context(tc.tile_pool(name="hT", bufs=1))
    hT = [
        hpool.tile([128, 16 * S], F8, name=f"hT{t}", tag=f"hT{t}") for t in range(2)
    ]  # hi, lo
    hTv = [t.rearrange("p (t s) -> p t s", t=16) for t in hT]

    bigp = mid.enter_context(tc.tile_pool(name="big", bufs=1, side="right"))
    kT = bigp.tile([128, NKV * S], BF16, name="kT", tag="kT")  # dual-slot
    va = [bigp.tile([128, 8 * 65], BF16, name=f"va{st}", tag=f"va{st}") for st in range(8)]
    qrp = mid.enter_context(tc.tile_pool(name="rope", bufs=2))
    qpool = mid.enter_context(tc.tile_pool(name="qtile", bufs=4))
    q_tiles = {}
    wqpool = mid.enter_context(tc.tile_pool(name="wq", bufs=3))

    wq_tiles = {}  # bq -> (hi view, lo view)

    def issue_wq(bq):
        vs = []
        for src, nm in ((wqh, "h"), (wql, "l")):
            w = wqpool.tile([128, 16 * 128], F8, name=f"wq{nm}", tag=f"wq{nm}")
            nc.sync.dma_start(w.rearrange("p (t f) -> p t f", t=16), src[bq])
            vs.append(w.rearrange("p (t f) -> p t f", t=16))
        wq_tiles[bq] = vs

    # ============ Phase V+K: V and K projections + K RoPE ============
    with tc.tile_pool(name="wk", bufs=1) as wkp:
        with tc.tile_pool(name="wv", bufs=1) as wvp, tc.tile_pool(
            name="vkpsum", bufs=4, space="PSUM"
        ) as vks:
            wv_t = []
            for srcv, nmv in ((wvh, "wvh"), (wvl, "wvl")):
                wv_ = wvp.tile([128, 16 * 512], F8, name=nmv, tag=nmv)
                nc.gpsimd.dma_start(out=wv_.rearrange("p (t f) -> p t f", t=16), in_=srcv[:])
                wv_t.append(wv_.rearrange("p (t f) -> p t f", t=16))
            hTd = hth.rearrange("(t p) s -> p t s", p=128)
            nc.sync.dma_start(hTv[0][:, 0:8], hTd[:, 0:8])
            nc.scalar.dma_start(hTv[0][:, 8:16], hTd[:, 8:16])
            wk_t = []
            for srck, nmk in ((wkh, "wkh"), (wkl, "wkl")):
                wk_ = wkp.tile([128, 16 * 512], F8, name=nmk, tag=nmk)
                nc.scalar.dma_start(wk_.rearrange("p (t f) -> p t f", t=16), srck[:])
                wk_t.append(wk_.rearrange("p (t f) -> p t f", t=16))
            hLd = htl.rearrange("(t p) s -> p t s", p=128)
            nc.sync.dma_start(hTv[1][:, 0:8], hLd[:, 0:8])
            nc.scalar.dma_start(hTv[1][:, 8:16], hLd[:, 8:16])
            nc.gpsimd.dma_start(out=tri_t[:], in_=trid[:])
            nc.gpsimd.dma_start(out=ident_t[:], in_=identd[:])
            nc.gpsimd.dma_start(out=cos_t[:], in_=cosd[:])
            nc.gpsimd.dma_start(out=sgn_t[:], in_=sgnd[:])
            issue_wq(0)
            issue_wq(1)
            for st in range(8):
                ps = vks.tile([128, 512], F32, name="vp", tag="vkp")
                n = 0
                for (a, b) in TERMS:
                    for j in range(8):
                        nc.tensor.matmul(
                            ps[:],
                            hTv[a][:, 2 * j : 2 * j + 2, st * 128 : (st + 1) * 128],
                            wv_t[b][:, 2 * j : 2 * j + 2, :],
                            start=(n == 0),
                            stop=(n == 23),
                            perf_mode=DR,
                        )
                        n += 1
                va3 = va[st].rearrange("p (g c) -> p g c", c=65)
                nc.scalar.activation(
                    va3[:, :, 0:64],
                    ps[:].rearrange("p (g c) -> p g c", c=64),
                    AF.Copy,
                    scale=1.0 / (WS * HS),
                )
                nc.gpsimd.memset(va3[:, :, 64:65], 1.0)
            for ft in range(4):
                for ih in range(2):
                    ps = vks.tile([128, 512], F32, name="kp", tag="vkp")
                    n = 0
                    for (a, b) in TERMS:
                        for j in range(8):
                            nc.tensor.matmul(
                                ps[:],
                                wk_t[b][:, 2 * j : 2 * j + 2, ft * 128 : (ft + 1) * 128],
                                hTv[a][:, 2 * j : 2 * j + 2, ih * 512 : (ih + 1) * 512],
                                start=(n == 0),
                                stop=(n == 23),
                                perf_mode=DR,
                            )
                            n += 1
                    sl = slice(ih * 512, (ih + 1) * 512)
                    kfin = qrp.tile([128, 512], BF16, name="kfin", tag="kfin")
                    _rope(nc, qrp, ps, cos_t[:, sl], sgn_t[:, sl], kfin[:], cp_eng=nc.scalar)
                    b0, b1 = 2 * ft, 2 * ft + 1
                    o0 = b0 * S + ih * 512
                    o1 = b1 * S + ih * 512
                    nc.sync.dma_start(kT[0:64, o0 : o0 + 512], kfin[0:64, :])
                    nc.scalar.dma_start(kT[64:128, o0 : o0 + 512], kfin[0:64, :])
                    nc.sync.dma_start(kT[64:128, o1 : o1 + 512], kfin[64:128, :])
                    nc.scalar.dma_start(kT[0:64, o1 : o1 + 512], kfin[64:128, :])

    # ============ Pipelined: Q projection / attention / transposes ==========
    P1W = mid.enter_context(tc.tile_pool(name="P1W", bufs=2, space="PSUM"))
    P1N = mid.enter_context(tc.tile_pool(name="P1N", bufs=2, space="PSUM"))
    pvp = mid.enter_context(tc.tile_pool(name="pv", bufs=1, space="PSUM"))
    exp_p = mid.enter_context(tc.tile_pool(name="ex", bufs=5))
    qdp = mid.enter_context(tc.tile_pool(name="qd", bufs=6))
    rdp = mid.enter_context(tc.tile_pool(name="rden", bufs=3))
    qd_tiles = {}  # (bq, hs) -> qd tile

    def q_proj_half(bq, ih, wv_):
        """One [128, 512] half of the Q projection for tile bq — PE filler."""
        if ih == 0:
            q_tiles[bq] = qpool.tile([128, S], BF16, name="qt", tag="qt")
        ps = P1N.tile([128, 512], F32, name="qp", tag="P1N")
        n = 0
        for (a, b) in TERMS:
            for j in range(8):
                nc.tensor.matmul(
                    ps[:],
                    wv_[b][:, 2 * j : 2 * j + 2, :],
                    hTv[a][:, 2 * j : 2 * j + 2, ih * 512 : (ih + 1) * 512],
                    start=(n == 0),
                    stop=(n == 23),
                    perf_mode=DR,
                )
                n += 1
        sl = slice(ih * 512, (ih + 1) * 512)
        _rope(nc, qrp, ps, cos_t[:, sl], sgn_t[:, sl],
              q_tiles[bq][:, ih * 512 : (ih + 1) * 512])

    def pvoff(it):
        return 65 * it if it < 4 else 512 + 65 * (it - 4)

    def qk(h, jt):
        """QK matmuls for (head, key-tile jt) + diag mask + one exp."""
        g = h // 4
        slot = 64 * (h % 2)
        lo = 128 * jt
        ex = exp_p.tile([128, 1024], BF16, name="ex", tag="ex")
        kap = kT[slot : slot + 64, g * S + lo : g * S + lo + 128]
        qap = q_tiles[h // 2][slot : slot + 64, :]
        if jt < 4:
            sc = P1W.tile([128, 1024], F32, name="scW", tag="P1W")
            nc.tensor.matmul(sc[:, lo:512], kap, qap[:, lo:512],
                             start=True, stop=False, skip_group_check=True)
            nc.tensor.matmul(sc[:, lo : lo + 128], ident_t[:], tri_t[:],
                             start=False, stop=True, skip_group_check=True)
            nc.tensor.matmul(sc[:, 512:1024], kap, qap[:, 512:1024],
                             start=True, stop=True, skip_group_check=True)
            nc.scalar.activation(ex[:, lo:1024], sc[:, lo:1024], AF.Exp, scale=0.125)
        else:
            sc = P1N.tile([128, 512], F32, name="scN", tag="P1N")
            nc.tensor.matmul(sc[:, 0 : 1024 - lo], kap, qap[:, lo:1024],
                             start=True, stop=False, skip_group_check=True)
            nc.tensor.matmul(sc[:, 0:128], ident_t[:], tri_t[:],
                             start=False, stop=True, skip_group_check=True)
            nc.scalar.activation(ex[:, lo:1024], sc[:, 0 : 1024 - lo], AF.Exp, scale=0.125)
        return ex

    def pv_all(h, jt, ex, pv):
        g = h // 4
        vag = va[jt].rearrange("p (g c) -> p g c", c=65)[:, g, :]
        for it in range(jt, 8):
            nc.tensor.matmul(
                pv[:, pvoff(it) : pvoff(it) + 65],
                ex[:, it * 128 : (it + 1) * 128], vag,
                start=(jt == 0 and it in (0, 4)),
                stop=((jt == 3 and it == 3) or (jt == 7 and it == 7)),
                skip_group_check=True,
            )

    def norm(pv, qd, rden):
        pvb0 = pv[:, 0:260].rearrange("p (b c) -> p b c", c=65)
        pvb1 = pv[:, 512:772].rearrange("p (b c) -> p b c", c=65)
        nc.vector.reciprocal_approx_fast(rden[:, 0:4], pvb0[:, :, 64:65])
        nc.vector.reciprocal_approx_fast(rden[:, 4:8], pvb1[:, :, 64:65])
        for it in range(8):
            nc.vector.tensor_scalar_mul(
                qd[:, it * 64 : (it + 1) * 64],
                pv[:, pvoff(it) : pvoff(it) + 64],
                rden[:, it : it + 1],
            )

    def transposes(bq, hs):
        """PE transposes of qd back to [d, q] layout + Pool copies into attT.

        Output PSUM is borrowed from the wide score pool via bitcast."""
        tpw = P1W.tile([128, 1024], F32, name="tpw", tag="P1W")
        qd = qd_tiles.pop((bq, hs))
        for it in range(8):
            dst = tpw[hs * 64 : hs * 64 + 64, it * 64 : (it + 1) * 64].bitcast(BF16)
            nc.tensor.transpose(
                dst,
                qd[:, it * 64 : (it + 1) * 64],
                ident_t[:],
                tile_position=(0, hs * 64),
            )
            nc.vector.tensor_copy(
                attT[hs * 64 : hs * 64 + 64, bq * S + it * 128 : bq * S + (it + 1) * 128],
                dst,
            )

    def attn_head(bq, hs, qfill, tfill):
        """Attention for head 2bq+hs; QK runs two key-tiles ahead of PV.

        qfill: Q-projection matmuls as PE filler after QK(0).
        tfill: deferred transposes, emitted after QK(1) so their PSUM
        allocation (shared with the wide score pool) waits on an exp that
        is already on the critical path."""
        h = 2 * bq + hs
        pv = pvp.tile([128, 772], F32, name="pv", tag="pv")
        qd = qdp.tile([128, 512], BF16, name="qd", tag="qd")
        rden = rdp.tile([128, 8], F32, name="rden", tag="rden")
        qd_tiles[(bq, hs)] = qd
        exs = {}
        exs[0] = qk(h, 0)
        if qfill is not None:
            qfill()
        exs[1] = qk(h, 1)
        if tfill is not None:
            tfill()
        for jt in range(8):
            if jt + 2 < 8:
                exs[jt + 2] = qk(h, jt + 2)
            pv_all(h, jt, exs.pop(jt), pv)
        norm(pv, qd, rden)

    woc = []

    def issue_wo(ho):
        w = wop.tile([128, 16 * 512], BF16, name="woc", tag="woc")
        nc.sync.dma_start(w.rearrange("p (t f) -> p t f", t=16), wob[ho])
        woc.append(w.rearrange("p (t f) -> p t f", t=16))

    wq_views = {}
    for i in range(17):
        if i + 2 < 16:
            issue_wq(i + 2)
        wq_views.update(wq_tiles)
        wq_tiles.clear()
        if i == 0:
            for bq in (0, 1):
                for ih in range(2):
                    q_proj_half(bq, ih, wq_views[bq])
            continue
        for hs in range(2):
            if i + 1 < 16:
                fill = (lambda bq=i + 1, ih=hs: q_proj_half(bq, ih, wq_views[bq]))
            else:
                fill = None
            tfill = (lambda bq=i - 2, h2=hs: transposes(bq, h2)) if i >= 2 else None
            attn_head(i - 1, hs, fill, tfill)
        if i == 14:
            issue_wo(0)
        if i == 15:
            issue_wo(1)
    for hs in range(2):
        transposes(15, hs)

    mid.close()  # free hT, kT, q, va, loop pools

    # ================= Phase O: O projection =================
    with tc.tile_pool(
        name="opsum", bufs=4, space="PSUM"
    ) as ops, tc.tile_pool(name="osb", bufs=4) as osbp:
        for ho in range(4):
            if ho + 2 < 4:
                issue_wo(ho + 2)
            for st in range(8):
                ps = ops.tile([128, 512], F32, name="op", tag="op")
                for t in range(16):
                    nc.tensor.matmul(
                        ps[:],
                        attT[:, t * S + st * 128 : t * S + st * 128 + 128],
                        woc[ho][:, t, :],
                        start=(t == 0),
                        stop=(t == 15),
                    )
                ob = osbp.tile([128, 512], F32, name="ob", tag="ob")
                nc.scalar.copy(ob[:], ps[:])
                eng = nc.sync if st % 2 == 0 else nc.gpsimd
                eng.dma_start(
                    out=outd[st * 128 : (st + 1) * 128, ho * 512 : (ho + 1) * 512],
                    in_=ob[:],
                )


def _build(niter=1):
    nc = bacc.Bacc(None, target_bir_lowering=False)
    hth = nc.declare_dram_parameter("hidT_hi", [H, S], F8, isOutput=False)
    htl = nc.declare_dram_parameter("hidT_lo", [H, S], F8, isOutput=False)
    wqh = nc.declare_dram_parameter("wq_hi", [16, 128, 16, 128], F8, isOutput=False)
    wql = nc.declare_dram_parameter("wq_lo", [16, 128, 16, 128], F8, isOutput=False)
    wkh = nc.declare_dram_parameter("wk_hi", [128, 16, 512], F8, isOutput=False)
    wkl = nc.declare_dram_parameter("wk_lo", [128, 16, 512], F8, isOutput=False)
    wvh = nc.declare_dram_parameter("wv_hi", [128, 16, 512], F8, isOutput=False)
    wvl = nc.declare_dram_parameter("wv_lo", [128, 16, 512], F8, isOutput=False)
    wob = nc.declare_dram_parameter("wo_b", [4, 128, 16, 512], BF16, isOutput=False)
    cosd = nc.declare_dram_parameter("rope_cos", [128, S], F32, isOutput=False)
    sgnd = nc.declare_dram_parameter("rope_sgn", [128, S], F32, isOutput=False)
    trid = nc.declare_dram_parameter("tri_mask", [128, 128], BF16, isOutput=False)
    identd = nc.declare_dram_parameter("ident_b", [128, 128], BF16, isOutput=False)
    outd = nc.declare_dram_parameter("out", [S, H], F32, isOutput=True)
    tensors = (hth, htl, wqh, wql, wkh, wkl, wvh, wvl, wob, cosd, sgnd, trid, identd, outd)

    with tile.TileContext(nc) as tc:
        for _ in range(niter):
            with contextlib.ExitStack() as ctx:
                _body(nc, tc, ctx, tensors)
    nc.compile()
    return nc


_CACHE = {}


def _get_nc(niter=1):
    if niter not in _CACHE:
        _CACHE[niter] = _build(niter)
    return _CACHE[niter]


def _hi_lo(x, scale):
    xs = np.asarray(x, np.float32) * scale
    hi = xs.astype(E4)
    lo = (xs - hi.astype(np.float32)).astype(E4)
    return hi, lo


def _in_maps(inputs):
    cos128, sgn128, tri, ident = _host_tables()
    wq_h, wq_l = _hi_lo(inputs["Wq"], WS)  # [2048, 2048]
    wk_h, wk_l = _hi_lo(inputs["Wk"], WS)  # [2048, 512]
    wv_h, wv_l = _hi_lo(inputs["Wv"], WS)
    base = {
        "wq_hi": np.ascontiguousarray(
            wq_h.reshape(16, 128, 16, 128).transpose(2, 1, 0, 3)),
        "wq_lo": np.ascontiguousarray(
            wq_l.reshape(16, 128, 16, 128).transpose(2, 1, 0, 3)),
        "wk_hi": np.ascontiguousarray(wk_h.reshape(16, 128, 512).transpose(1, 0, 2)),
        "wk_lo": np.ascontiguousarray(wk_l.reshape(16, 128, 512).transpose(1, 0, 2)),
        "wv_hi": np.ascontiguousarray(wv_h.reshape(16, 128, 512).transpose(1, 0, 2)),
        "wv_lo": np.ascontiguousarray(wv_l.reshape(16, 128, 512).transpose(1, 0, 2)),
        "wo_b": np.ascontiguousarray(
            np.asarray(inputs["Wo"], np.float32).astype(BF)
            .reshape(16, 128, 4, 512).transpose(2, 1, 0, 3)),
        "rope_cos": cos128,
        "rope_sgn": sgn128,
        "tri_mask": tri,
        "ident_b": ident,
    }
    hidden = np.asarray(inputs["hidden_states"], np.float32)
    maps = []
    for b in range(B):
        h_h, h_l = _hi_lo(hidden[b].T, HS)  # [2048, 1024]
        maps.append(dict(base, hidT_hi=np.ascontiguousarray(h_h),
                         hidT_lo=np.ascontiguousarray(h_l)))
    return maps


def kernel(**inputs):
    nc = _get_nc(1)
    res = run_bass_kernel_spmd(nc, _in_maps(inputs), core_ids=list(range(8)))
    return np.stack([res.results[i]["out"] for i in range(B)]).astype(np.float32)


# revision 32
# speedup vs baseline: 1.0729x; 1.0729x over previous
"""Trainium2 Bass kernel for GQA attention (B=8, S=1024, H=2048, 32 Q / 8 KV heads, D=64).

Data-parallel over batch: one batch element per NeuronCore, weights replicated,
zero collectives. Host-side prep (numpy): hidden transposed to [H, S] and
decomposed into fp8e4 hi+lo at scale 16; Wq/Wk/Wv decomposed into fp8e4 hi+lo
at scale 512; Wo cast bf16; RoPE tables pre-scaled by 1/(16*512).

Device pipeline per core:
  1. Q/K/V projections as fp8 DoubleRow matmuls (2 K-chunks per pass,
     0.5 cycles/row), 3 error-compensated terms: Hh*Wh + Hh*Wl + Hl*Wh.
  2. RoPE via partition-shift SBUF DMAs + DVE/Pool mul-add (tables carry the
     fp8 descale), q in a rolling SBUF pool bf16, k duplicated into both
     64-partition slots of kT.
  3. Attention per head in scoresT [keys, queries] layout: causal-range QK
     matmuls with the diagonal NEG mask accumulated via identity matmul,
     one exp per key-tile on ScalarE (only Act work), PV in [query, d]
     layout (65-col matmuls, ones-column denominator), per-partition
     normalization (reciprocal + tensor_scalar_mul), PE-transpose back to
     [d, query] into attT bf16 (transpose PSUM borrowed from the wide
     score pool via bitcast).
  4. Software pipelining: step i runs attention(i-1) with Q-proj(i+1)
     DoubleRow matmuls as head-start PE filler and QK emitted two key-tiles
     ahead of PV; transposes(i-2) at step front; Wq streamed per-step; Wo
     prefetched during the last steps.
  5. O-projection bf16 from attT.
"""

import contextlib

import numpy as np
import ml_dtypes

import concourse.bass as bass
import concourse.tile as tile
from concourse import bacc, mybir
from concourse.bass_utils import run_bass_kernel_spmd

B, S, H = 8, 1024, 2048
NQ, NKV, D = 32, 8, 64
F32 = mybir.dt.float32
BF16 = mybir.dt.bfloat16
F8 = mybir.dt.float8e4
DR = mybir.MatmulPerfMode.DoubleRow
AF = mybir.ActivationFunctionType
WS = 512.0  # fp8 weight scale
HS = 16.0  # fp8 hidden scale
E4 = ml_dtypes.float8_e4m3
BF = ml_dtypes.bfloat16


def _host_tables():
    inv = 1.0 / (10000.0 ** (np.arange(0, D, 2, dtype=np.float64) / D))  # [32]
    fr = np.arange(S, dtype=np.float64)[:, None] * inv[None, :]  # [S, 32]
    cos = np.cos(fr).T  # [32, S]
    sin = np.sin(fr).T
    cosT = np.concatenate([cos, cos], 0)  # [64, S]
    sgnT = np.concatenate([-sin, sin], 0)  # [64, S]
    cos128 = (np.concatenate([cosT, cosT], 0) / (WS * HS)).astype(np.float32)
    sgn128 = (np.concatenate([sgnT, sgnT], 0) / (WS * HS)).astype(np.float32)
    p = np.arange(128)[:, None]  # key row
    c = np.arange(128)[None, :]  # query col
    tri = np.where(c >= p, 0.0, -1.0e30).astype(np.float32).astype(BF)  # additive mask
    ident = np.eye(128, dtype=np.float32).astype(BF)
    return cos128, sgn128, tri, ident


def _rope(nc, rp, ps, cos_sl, sgn_sl, out_sl, cp_eng=None):
    """psum [128,512] (scaled qT/kT tile) -> RoPE applied, written to out_sl (bf16).

    GPSIMD cannot access PSUM, so the raw copy runs on cp_eng (DVE/Act)."""
    raw = rp.tile([128, 512], F32, name="rope_raw", tag="rope_raw")
    if cp_eng is None:
        nc.vector.tensor_copy(raw[:], ps[:])
    else:
        cp_eng.copy(raw[:], ps[:])
    sh = rp.tile([128, 512], F32, name="rope_sh", tag="rope_sh")
    for a in range(4):  # partition quarter a reads quarter a^1 (p -> p xor 32)
        sc = (a ^ 1) * 32
        eng = nc.sync if a % 2 == 0 else nc.gpsimd
        eng.dma_start(out=sh[a * 32 : (a + 1) * 32, :], in_=raw[sc : sc + 32, :])
    tmp = rp.tile([128, 512], F32, name="rope_tmp", tag="rope_tmp")
    nc.vector.tensor_mul(tmp[:], raw[:], cos_sl)
    rot = rp.tile([128, 512], F32, name="rope_rot", tag="rope_rot")
    nc.gpsimd.tensor_mul(rot[:], sh[:], sgn_sl)
    nc.vector.tensor_add(out_sl, tmp[:], rot[:])


# DoubleRow 3-term schedule: (hidden term, weight term) with hi=0, lo=1.
# Ordered so hi-only terms run first (their DMAs land earlier).
TERMS = [(0, 0), (1, 0), (0, 1)]


def _body(nc, tc, ctx, tensors):
    (hth, htl, wqh, wql, wkh, wkl, wvh, wvl, wob, cosd, sgnd, trid, identd, outd) = tensors

    # ---- constants (tiles now, DMAs emitted after the wv prefetch) ----
    cpool = ctx.enter_context(tc.tile_pool(name="const", bufs=1))
    tri_t = cpool.tile([128, 128], BF16, name="tri", tag="tri")
    ident_t = cpool.tile([128, 128], BF16, name="ident", tag="ident")
    cos_t = cpool.tile([128, S], F32, name="cos", tag="cos")
    sgn_t = cpool.tile([128, S], F32, name="sgn", tag="sgn")

    # ---- persistent SBUF tensors ----
    attp = ctx.enter_context(tc.tile_pool(name="attTp", bufs=1, side="right"))
    attT = attp.tile([128, 16 * S], BF16, name="attT", tag="attT")
    wop = ctx.enter_context(tc.tile_pool(name="wo", bufs=2))

    # everything below `mid` is freed before the O projection
    mid = contextlib.ExitStack()
    hpool = mid.enter_context(tc.tile_pool(name="hT", bufs=1))
    hT = [
        hpool.tile([128, 16 * S], F8, name=f"hT{t}", tag=f"hT{t}") for t in range(2)
    ]  # hi, lo
    hTv = [t.rearrange("p (t s) -> p t s", t=16) for t in hT]

    bigp = mid.enter_context(tc.tile_pool(name="big", bufs=1, side="right"))
    kT = bigp.tile([128, NKV * S], BF16, name="kT", tag="kT")  # dual-slot
    va = [bigp.tile([128, 8 * 65], BF16, name=f"va{st}", tag=f"va{st}") for st in range(8)]
    qrp = mid.enter_context(tc.tile_pool(name="rope", bufs=2))
    qpool = mid.enter_context(tc.tile_pool(name="qtile", bufs=4))
    q_tiles = {}
    wqpool = mid.enter_context(tc.tile_pool(name="wq", bufs=3))

    wq_tiles = {}  # bq -> (hi view, lo view)

    def issue_wq(bq):
        vs = []
        for src, nm in ((wqh, "h"), (wql, "l")):
            w = wqpool.tile([128, 16 * 128], F8, name=f"wq{nm}", tag=f"wq{nm}")
            nc.sync.dma_start(w.rearrange("p (t f) -> p t f", t=16), src[bq])
            vs.append(w.rearrange("p (t f) -> p t f", t=16))
        wq_tiles[bq] = vs

    # ============ Phase V+K: V and K projections + K RoPE ============
    with tc.tile_pool(name="wk", bufs=1) as wkp:
        with tc.tile_pool(name="wv", bufs=1) as wvp, tc.tile_pool(
            name="vkpsum", bufs=4, space="PSUM"
        ) as vks:
            wv_t = []
            for srcv, nmv in ((wvh, "wvh"), (wvl, "wvl")):
                wv_ = wvp.tile([128, 16 * 512], F8, name=nmv, tag=nmv)
                nc.gpsimd.dma_start(out=wv_.rearrange("p (t f) -> p t f", t=16), in_=srcv[:])
                wv_t.append(wv_.rearrange("p (t f) -> p t f", t=16))
            hTd = hth.rearrange("(t p) s -> p t s", p=128)
            nc.sync.dma_start(hTv[0][:, 0:8], hTd[:, 0:8])
            nc.scalar.dma_start(hTv[0][:, 8:16], hTd[:, 8:16])
            wk_t = []
            for srck, nmk in ((wkh, "wkh"), (wkl, "wkl")):
                wk_ = wkp.tile([128, 16 * 512], F8, name=nmk, tag=nmk)
                nc.scalar.dma_start(wk_.rearrange("p (t f) -> p t f", t=16), srck[:])
                wk_t.append(wk_.rearrange("p (t f) -> p t f", t=16))
            hLd = htl.rearrange("(t p) s -> p t s", p=128)
            nc.sync.dma_start(hTv[1][:, 0:8], hLd[:, 0:8])
            nc.scalar.dma_start(hTv[1][:, 8:16], hLd[:, 8:16])
            nc.gpsimd.dma_start(out=tri_t[:], in_=trid[:])
            nc.gpsimd.dma_start(out=ident_t[:], in_=identd[:])
            nc.gpsimd.dma_start(out=cos_t[:], in_=cosd[:])
            nc.gpsimd.dma_start(out=sgn_t[:], in_=sgnd[:])
            issue_wq(0)
            issue_wq(1)
            for st in range(8):
                ps = vks.tile([128, 512], F32, name="vp", tag="vkp")
                n = 0
                for (a, b) in TERMS:
                    for j in range(8):
                        nc.tensor.matmul(
                            ps[:],
                            hTv[a][:, 2 * j : 2 * j + 2, st * 128 : (st + 1) * 128],
                            wv_t[b][:, 2 * j : 2 * j + 2, :],
                            start=(n == 0),
                            stop=(n == 23),
                            perf_mode=DR,
                        )
                        n += 1
                va3 = va[st].rearrange("p (g c) -> p g c", c=65)
                nc.scalar.activation(
                    va3[:, :, 0:64],
                    ps[:].rearrange("p (g c) -> p g c", c=64),
                    AF.Copy,
                    scale=1.0 / (WS * HS),
                )
                nc.gpsimd.memset(va3[:, :, 64:65], 1.0)
            for ft in range(4):
                for ih in range(2):
                    ps = vks.tile([128, 512], F32, name="kp", tag="vkp")
                    n = 0
                    for (a, b) in TERMS:
                        for j in range(8):
                            nc.tensor.matmul(
                                ps[:],
                                wk_t[b][:, 2 * j : 2 * j + 2, ft * 128 : (ft + 1) * 128],
                                hTv[a][:, 2 * j : 2 * j + 2, ih * 512 : (ih + 1) * 512],
                                start=(n == 0),
                                stop=(n == 23),
                                perf_mode=DR,
                            )
                            n += 1
                    sl = slice(ih * 512, (ih + 1) * 512)
                    kfin = qrp.tile([128, 512], BF16, name="kfin", tag="kfin")
                    _rope(nc, qrp, ps, cos_t[:, sl], sgn_t[:, sl], kfin[:], cp_eng=nc.scalar)
                    b0, b1 = 2 * ft, 2 * ft + 1
                    o0 = b0 * S + ih * 512
                    o1 = b1 * S + ih * 512
                    nc.sync.dma_start(kT[0:64, o0 : o0 + 512], kfin[0:64, :])
                    nc.scalar.dma_start(kT[64:128, o0 : o0 + 512], kfin[0:64, :])
                    nc.sync.dma_start(kT[64:128, o1 : o1 + 512], kfin[64:128, :])
                    nc.scalar.dma_start(kT[0:64, o1 : o1 + 512], kfin[64:128, :])

    # ============ Pipelined: Q projection / attention / transposes ==========
    P1W = mid.enter_context(tc.tile_pool(name="P1W", bufs=2, space="PSUM"))
    P1N = mid.enter_context(tc.tile_pool(name="P1N", bufs=2, space="PSUM"))
    pvp = mid.enter_context(tc.tile_pool(name="pv", bufs=1, space="PSUM"))
    exp_p = mid.enter_context(tc.tile_pool(name="ex", bufs=5))
    qdp = mid.enter_context(tc.tile_pool(name="qd", bufs=6))
    rdp = mid.enter_context(tc.tile_pool(name="rden", bufs=3))
    qd_tiles = {}  # (bq, hs) -> qd tile

    def q_proj_half(bq, ih, wv_):
        """One [128, 512] half of the Q projection for tile bq — PE filler."""
        if ih == 0:
            q_tiles[bq] = qpool.tile([128, S], BF16, name="qt", tag="qt")
        ps = P1N.tile([128, 512], F32, name="qp", tag="P1N")
        n = 0
        for (a, b) in TERMS:
            for j in range(8):
                nc.tensor.matmul(
                    ps[:],
                    wv_[b][:, 2 * j : 2 * j + 2, :],
                    hTv[a][:, 2 * j : 2 * j + 2, ih * 512 : (ih + 1) * 512],
                    start=(n == 0),
                    stop=(n == 23),
                    perf_mode=DR,
                )
                n += 1
        sl = slice(ih * 512, (ih + 1) * 512)
        _rope(nc, qrp, ps, cos_t[:, sl], sgn_t[:, sl],
              q_tiles[bq][:, ih * 512 : (ih + 1) * 512])

    def pvoff(it):
        return 65 * it if it < 4 else 512 + 65 * (it - 4)

    def qk(h, jt):
        """QK matmuls for (head, key-tile jt) + diag mask + one exp."""
        g = h // 4
        slot = 64 * (h % 2)
        lo = 128 * jt
        ex = exp_p.tile([128, 1024], BF16, name="ex", tag="ex")
        kap = kT[slot : slot + 64, g * S + lo : g * S + lo + 128]
        qap = q_tiles[h // 2][slot : slot + 64, :]
        if jt < 4:
            sc = P1W.tile([128, 1024], F32, name="scW", tag="P1W")
            nc.tensor.matmul(sc[:, lo:512], kap, qap[:, lo:512],
                             start=True, stop=False, skip_group_check=True)
            nc.tensor.matmul(sc[:, lo : lo + 128], ident_t[:], tri_t[:],
                             start=False, stop=True, skip_group_check=True)
            nc.tensor.matmul(sc[:, 512:1024], kap, qap[:, 512:1024],
                             start=True, stop=True, skip_group_check=True)
            nc.scalar.activation(ex[:, lo:1024], sc[:, lo:1024], AF.Exp, scale=0.125)
        else:
            sc = P1N.tile([128, 512], F32, name="scN", tag="P1N")
            nc.tensor.matmul(sc[:, 0 : 1024 - lo], kap, qap[:, lo:1024],
                             start=True, stop=False, skip_group_check=True)
            nc.tensor.matmul(sc[:, 0:128], ident_t[:], tri_t[:],
                             start=False, stop=True, skip_group_check=True)
            nc.scalar.activation(ex[:, lo:1024], sc[:, 0 : 1024 - lo], AF.Exp, scale=0.125)
        return ex

    def pv_all(h, jt, ex, pv):
        g = h // 4
        vag = va[jt].rearrange("p (g c) -> p g c", c=65)[:, g, :]
        for it in range(jt, 8):
            nc.tensor.matmul(
                pv[:, pvoff(it) : pvoff(it) + 65],
                ex[:, it * 128 : (it + 1) * 128], vag,
                start=(jt == 0 and it in (0, 4)),
                stop=((jt == 3 and it == 3) or (jt == 7 and it == 7)),
                skip_group_check=True,
            )

    def norm(pv, qd, rden):
        pvb0 = pv[:, 0:260].rearrange("p (b c) -> p b c", c=65)
        pvb1 = pv[:, 512:772].rearrange("p (b c) -> p b c", c=65)
        nc.vector.reciprocal_approx_fast(rden[:, 0:4], pvb0[:, :, 64:65])
        nc.vector.reciprocal_approx_fast(rden[:, 4:8], pvb1[:, :, 64:65])
        for it in range(8):
            nc.vector.tensor_scalar_mul(
                qd[:, it * 64 : (it + 1) * 64],
                pv[:, pvoff(it) : pvoff(it) + 64],
                rden[:, it : it + 1],
            )

    def transposes(bq, hs):
        """PE transposes of qd back to [d, q] layout + DVE copies into attT.

        Output PSUM is one narrow-pool tile (8 x 64 f32 cols, bitcast bf16)."""
        tpn = P1N.tile([128, 512], F32, name="tpn", tag="P1N")
        qd = qd_tiles.pop((bq, hs))
        for it in range(8):
            dst = tpn[hs * 64 : hs * 64 + 64, it * 64 : (it + 1) * 64].bitcast(BF16)
            nc.tensor.transpose(
                dst,
                qd[:, it * 64 : (it + 1) * 64],
                ident_t[:],
                tile_position=(0, hs * 64),
            )
            nc.vector.tensor_copy(
                attT[hs * 64 : hs * 64 + 64, bq * S + it * 128 : bq * S + (it + 1) * 128],
                dst,
            )

    def attn_head(bq, hs, qfill, tfill):
        """Attention for head 2bq+hs; QK runs two key-tiles ahead of PV.

        qfill: Q-projection matmuls as PE filler after QK(0).
        tfill: deferred transposes, emitted after QK(1) so their PSUM
        allocation (shared with the wide score pool) waits on an exp that
        is already on the critical path."""
        h = 2 * bq + hs
        pv = pvp.tile([128, 772], F32, name="pv", tag="pv")
        qd = qdp.tile([128, 512], BF16, name="qd", tag="qd")
        rden = rdp.tile([128, 8], F32, name="rden", tag="rden")
        qd_tiles[(bq, hs)] = qd
        exs = {}
        exs[0] = qk(h, 0)
        if qfill is not None:
            qfill()
        exs[1] = qk(h, 1)
        if tfill is not None:
            tfill()
        for jt in range(8):
            if jt + 2 < 8:
                exs[jt + 2] = qk(h, jt + 2)
            pv_all(h, jt, exs.pop(jt), pv)
        norm(pv, qd, rden)

    woc = []

    def issue_wo(ho):
        w = wop.tile([128, 16 * 512], BF16, name="woc", tag="woc")
        nc.sync.dma_start(w.rearrange("p (t f) -> p t f", t=16), wob[ho])
        woc.append(w.rearrange("p (t f) -> p t f", t=16))

    wq_views = {}
    for i in range(17):
        if i + 2 < 16:
            issue_wq(i + 2)
        wq_views.update(wq_tiles)
        wq_tiles.clear()
        if i == 0:
            for bq in (0, 1):
                for ih in range(2):
                    q_proj_half(bq, ih, wq_views[bq])
            continue
        for hs in range(2):
            if i + 1 < 16:
                fill = (lambda bq=i + 1, ih=hs: q_proj_half(bq, ih, wq_views[bq]))
            else:
                fill = None
            tfill = (lambda bq=i - 2, h2=hs: transposes(bq, h2)) if i >= 2 else None
            attn_head(i - 1, hs, fill, tfill)
        if i == 14:
            issue_wo(0)
        if i == 15:
            issue_wo(1)
    for hs in range(2):
        transposes(15, hs)

    mid.close()  # free hT, kT, q, va, loop pools

    # ================= Phase O: O projection =================
    with tc.tile_pool(
        name="opsum", bufs=4, space="PSUM"
    ) as ops, tc.tile_pool(name="osb", bufs=4) as osbp:
        for ho in range(4):
            if ho + 2 < 4:
                issue_wo(ho + 2)
            for st in range(8):
                ps = ops.tile([128, 512], F32, name="op", tag="op")
                for t in range(16):
                    nc.tensor.matmul(
                        ps[:],
                        attT[:, t * S + st * 128 : t * S + st * 128 + 128],
                        woc[ho][:, t, :],
                        start=(t == 0),
                        stop=(t == 15),
                    )
                ob = osbp.tile([128, 512], F32, name="ob", tag="ob")
                nc.scalar.copy(ob[:], ps[:])
                eng = nc.sync if st % 2 == 0 else nc.gpsimd
                eng.dma_start(
                    out=outd[st * 128 : (st + 1) * 128, ho * 512 : (ho + 1) * 512],
                    in_=ob[:],
                )


def _build(niter=1):
    nc = bacc.Bacc(None, target_bir_lowering=False)
    hth = nc.declare_dram_parameter("hidT_hi", [H, S], F8, isOutput=False)
    htl = nc.declare_dram_parameter("hidT_lo", [H, S], F8, isOutput=False)
    wqh = nc.declare_dram_parameter("wq_hi", [16, 128, 16, 128], F8, isOutput=False)
    wql = nc.declare_dram_parameter("wq_lo", [16, 128, 16, 128], F8, isOutput=False)
    wkh = nc.declare_dram_parameter("wk_hi", [128, 16, 512], F8, isOutput=False)
    wkl = nc.declare_dram_parameter("wk_lo", [128, 16, 512], F8, isOutput=False)
    wvh = nc.declare_dram_parameter("wv_hi", [128, 16, 512], F8, isOutput=False)
    wvl = nc.declare_dram_parameter("wv_lo", [128, 16, 512], F8, isOutput=False)
    wob = nc.declare_dram_parameter("wo_b", [4, 128, 16, 512], BF16, isOutput=False)
    cosd = nc.declare_dram_parameter("rope_cos", [128, S], F32, isOutput=False)
    sgnd = nc.declare_dram_parameter("rope_sgn", [128, S], F32, isOutput=False)
    trid = nc.declare_dram_parameter("tri_mask", [128, 128], BF16, isOutput=False)
    identd = nc.declare_dram_parameter("ident_b", [128, 128], BF16, isOutput=False)
    outd = nc.declare_dram_parameter("out", [S, H], F32, isOutput=True)
    tensors = (hth, htl, wqh, wql, wkh, wkl, wvh, wvl, wob, cosd, sgnd, trid, identd, outd)

    with tile.TileContext(nc) as tc:
        for _ in range(niter):
            with contextlib.ExitStack() as ctx:
                _body(nc, tc, ctx, tensors)
    nc.compile()
    return nc


_CACHE = {}


def _get_nc(niter=1):
    if niter not in _CACHE:
        _CACHE[niter] = _build(niter)
    return _CACHE[niter]


def _hi_lo(x, scale):
    xs = np.asarray(x, np.float32) * scale
    hi = xs.astype(E4)
    lo = (xs - hi.astype(np.float32)).astype(E4)
    return hi, lo


def _in_maps(inputs):
    cos128, sgn128, tri, ident = _host_tables()
    wq_h, wq_l = _hi_lo(inputs["Wq"], WS)  # [2048, 2048]
    wk_h, wk_l = _hi_lo(inputs["Wk"], WS)  # [2048, 512]
    wv_h, wv_l = _hi_lo(inputs["Wv"], WS)
    base = {
        "wq_hi": np.ascontiguousarray(
            wq_h.reshape(16, 128, 16, 128).transpose(2, 1, 0, 3)),
        "wq_lo": np.ascontiguousarray(
            wq_l.reshape(16, 128, 16, 128).transpose(2, 1, 0, 3)),
        "wk_hi": np.ascontiguousarray(wk_h.reshape(16, 128, 512).transpose(1, 0, 2)),
        "wk_lo": np.ascontiguousarray(wk_l.reshape(16, 128, 512).transpose(1, 0, 2)),
        "wv_hi": np.ascontiguousarray(wv_h.reshape(16, 128, 512).transpose(1, 0, 2)),
        "wv_lo": np.ascontiguousarray(wv_l.reshape(16, 128, 512).transpose(1, 0, 2)),
        "wo_b": np.ascontiguousarray(
            np.asarray(inputs["Wo"], np.float32).astype(BF)
            .reshape(16, 128, 4, 512).transpose(2, 1, 0, 3)),
        "rope_cos": cos128,
        "rope_sgn": sgn128,
        "tri_mask": tri,
        "ident_b": ident,
    }
    hidden = np.asarray(inputs["hidden_states"], np.float32)
    maps = []
    for b in range(B):
        h_h, h_l = _hi_lo(hidden[b].T, HS)  # [2048, 1024]
        maps.append(dict(base, hidT_hi=np.ascontiguousarray(h_h),
                         hidT_lo=np.ascontiguousarray(h_l)))
    return maps


def kernel(**inputs):
    nc = _get_nc(1)
    res = run_bass_kernel_spmd(nc, _in_maps(inputs), core_ids=list(range(8)))
    return np.stack([res.results[i]["out"] for i in range(B)]).astype(np.float32)
